# revision 28
# baseline (speedup 1.0000x reference)
"""Trainium2 Bass kernel for nn_AttentionLayer (attention pooling).

reference:
    score  = tanh(x @ W + b)          # [B,T,U]
    logits = score @ u                # [B,T,1]
    a      = softmax(logits, axis=T)  # [B,T,1]
    ctx    = sum_t x * a              # [B,D]
    returns (ctx, a)

Sharding: data-parallel over batch B=64 -> 8 cores x 8 batches.
Per-core pipeline (per batch):
    1. DMA x[b] (4096,256) fp32 -> SBUF bf16 [128,32,256]  (t on partitions)
    2. PE-transpose 64 x [128,128] tiles -> xT bf16 [128d, 2c, 32n, 128t]
    3. scoreT[u=64, t] = W.T @ xT   (PE, accumulate 2 d-chunks, 512-wide)
    4. tanh+bias on ACT (PSUM->SBUF, bf16)
    5. logits[t=128,32] : per t-chunk matmul (lhsT=scoreT strip, rhs=u)
    6. exp+rowsum on ACT; total via PE ones-matmul; reciprocal on DVE;
       broadcast via PE; a = e * invS on DVE
    7. ctx = sum_t a[t]*x[t,:] : PE accumulation (lhsT=a col, rhs=x tile)
"""
import sys
sys.path.insert(0, "/opt/trn_rl_repo")
import numpy as np
from contextlib import ExitStack

import concourse.bass as bass
import concourse.bacc as bacc
import concourse.tile as tile
from concourse import mybir
from concourse._compat import with_exitstack
from concourse.bass_utils import run_bass_kernel_spmd
from concourse.masks import make_identity

AF = mybir.ActivationFunctionType
F32 = mybir.dt.float32
BF16 = mybir.dt.float16  # 16-bit compute dtype (fp16: 10-bit mantissa)

P = 128          # partitions
T = 4096         # time steps
D = 256          # feature dim
U = 64           # attention units
NT = T // P      # 32 t-chunks of 128
NBLK = T // 512  # 8 t-blocks of 512
BSH = 8          # batches per core
NCORES = 8


@with_exitstack
def _body(ctx: ExitStack, tc: tile.TileContext, x, W, bvec, u, out_ctx, out_attn):
    nc = tc.nc
    consts = ctx.enter_context(tc.tile_pool(name="consts", bufs=1))
    # x alive from load(b) until ctx(b), emitted 2 batches later
    xpool = ctx.enter_context(tc.tile_pool(name="x", bufs=4))
    x32pool = ctx.enter_context(tc.tile_pool(name="x32", bufs=2))
    xtpool = ctx.enter_context(tc.tile_pool(name="xt", bufs=2))
    spool = ctx.enter_context(tc.tile_pool(name="scoreT", bufs=2))
    small = ctx.enter_context(tc.tile_pool(name="small", bufs=6))
    tps = ctx.enter_context(tc.tile_pool(name="tps", bufs=4, space="PSUM"))
    sps = ctx.enter_context(tc.tile_pool(name="sps", bufs=2, space="PSUM"))
    cps = ctx.enter_context(tc.tile_pool(name="cps", bufs=1, space="PSUM"))
    mps = ctx.enter_context(tc.tile_pool(name="mps", bufs=1, space="PSUM"))

    ident = consts.tile([P, P], F32)
    make_identity(nc, ident)
    # plain fp32 loads (HWDGE) + on-chip casts: SWDGE cast-DMA avoided
    w32 = consts.tile([P, 2, U], F32)
    nc.sync.dma_start(out=w32, in_=W.rearrange("(c k) u -> k c u", k=P))
    w_sb = consts.tile([P, 2, U], BF16)
    nc.vector.tensor_copy(w_sb, w32)
    # u and b replicated into both 64-partition halves (for row/col tiling)
    u32 = consts.tile([P, 1], F32)
    nc.sync.dma_start(out=u32[0:U, :], in_=u)
    nc.sync.dma_start(out=u32[U:2 * U, :], in_=u)
    u_sb = consts.tile([P, 1], BF16)
    nc.vector.tensor_copy(u_sb, u32)
    b_sb = consts.tile([P, 1], F32)
    nc.sync.dma_start(out=b_sb[0:U, :], in_=bvec.rearrange("(u o) -> u o", o=1))
    nc.sync.dma_start(out=b_sb[U:2 * U, :], in_=bvec.rearrange("(u o) -> u o", o=1))
    ones_col = consts.tile([P, 1], F32)
    nc.vector.memset(ones_col, 1.0)
    ones_row = consts.tile([1, P], F32)
    nc.vector.memset(ones_row, 1.0)

    def emit_ctx(bi, x_lo, x_hi, a_bf):
        # ctx = sum_t a[t] * x[t,:]  (PE accumulate over 32 t-chunks)
        c_ps = cps.tile([1, D], F32)
        for n in range(NT):
            xt = x_lo[:, n, :] if n < NT // 2 else x_hi[:, n - NT // 2, :]
            nc.tensor.matmul(c_ps, a_bf[:, n:n + 1], xt,
                             start=(n == 0), stop=(n == NT - 1))
        ctx_sb = small.tile([1, D], F32)
        nc.scalar.activation(ctx_sb, c_ps, AF.Copy)
        nc.sync.dma_start(out=out_ctx[bi].rearrange("(o d) -> o d", o=1), in_=ctx_sb)

    pending = []  # (bi, x_sb, a_bf) awaiting ctx emission

    for bi in range(BSH):
        # 1. load x[bi] in two halves (earlier pipeline start) as plain fp32
        # (HWDGE), then cast to fp16 on ACT/DVE in quarters
        xre = x[bi].rearrange("(n p) d -> p n d", p=P)
        x_lo = xpool.tile([P, NT // 2, D], BF16, tag="xlo")
        x_hi = xpool.tile([P, NT // 2, D], BF16, tag="xhi")
        x32_lo = x32pool.tile([P, NT // 2, D], F32, tag="x32lo")
        x32_hi = x32pool.tile([P, NT // 2, D], F32, tag="x32hi")
        for h, (x16h, x32h, st) in enumerate([(x_lo, x32_lo, 0),
                                              (x_hi, x32_hi, NT // 2)]):
            nc.sync.dma_start(out=x32h, in_=xre[:, st:st + NT // 2, :])
            for q in range(2):
                sl = (slice(None), slice(q * (NT // 4), (q + 1) * (NT // 4)),
                      slice(None))
                if 2 * h + q == 3:
                    nc.vector.tensor_copy(x16h[sl], x32h[sl])
                else:
                    nc.scalar.activation(x16h[sl], x32h[sl], AF.Copy)

        def x32_tile(n):
            return (x32_lo[:, n, :] if n < NT // 2
                    else x32_hi[:, n - NT // 2, :])

        # 2+3+4. interleaved per 512-block: transpose the 16 fp32 [128,128]
        # tiles it needs (each into its own PSUM tile, copy+cast to fp16 on
        # ACT/DVE — the exact pattern validated by the 1-core probes), then
        # the score matmuls + tanh for that block.
        xT = xtpool.tile([P, 2, NT, P], BF16)
        scoreT = spool.tile([U, NBLK, 512], BF16)
        for blk in range(NBLK):
            for k in range(8):
                n, c = divmod(8 * blk + k, 2)
                tp = tps.tile([P, P], F32)
                nc.tensor.transpose(tp, x32_tile(n)[:, c * P:(c + 1) * P],
                                    ident)
                if k % 4 == 0:
                    nc.scalar.activation(xT[:, c, n, :], tp, AF.Copy)
                else:
                    nc.vector.tensor_copy(xT[:, c, n, :], tp)

            s_ps = sps.tile([U, 512], F32)
            for c in range(2):
                nc.tensor.matmul(s_ps, w_sb[:, c, :],
                                 xT[:, c, 4 * blk:4 * blk + 4, :],
                                 start=(c == 0), stop=(c == 1))
            nc.scalar.activation(scoreT[:, blk, :], s_ps, AF.Tanh,
                                 bias=b_sb[0:U, :])

        # 5. logits [128, 32]: column j = logits for t in [128j, 128j+128)
        l_ps = mps.tile([P, NT], F32, tag="misc")
        for j in range(NT):
            nc.tensor.matmul(l_ps[:, j:j + 1],
                             scoreT[:, j // 4, 128 * (j % 4):128 * (j % 4 + 1)],
                             u_sb[0:U, :], start=True, stop=True)

        # 6. softmax over all 4096 (no max-subtraction: |logits| <= 64*0.33*1)
        e_sb = small.tile([P, NT], F32)
        rowsum = small.tile([P, 1], F32)
        nc.scalar.activation(e_sb, l_ps, AF.Exp, accum_out=rowsum)
        s_ps1 = mps.tile([1, 1], F32, tag="misc")
        nc.tensor.matmul(s_ps1, rowsum, ones_col, start=True, stop=True)
        invS = small.tile([1, 1], F32)
        nc.vector.reciprocal(invS, s_ps1)
        ib_ps = mps.tile([P, 1], F32, tag="misc")
        nc.tensor.matmul(ib_ps, ones_row, invS, start=True, stop=True)
        invS_b = small.tile([P, 1], F32)
        nc.scalar.activation(invS_b, ib_ps, AF.Copy)
        a_sb = small.tile([P, NT], F32)
        nc.vector.tensor_scalar_mul(a_sb, e_sb, invS_b)
        # attn stored [P, NT] per batch (contiguous per-partition DMA);
        # host transposes to [T, 1] when unsharding.
        nc.sync.dma_start(out=out_attn[bi], in_=a_sb)

        # 7. ctx is emitted 2 batches late so the softmax tail never stalls
        # PE: ctx(b-2) matmuls fill PE while softmax(b) runs on ACT/DVE.
        a_bf = small.tile([P, NT], BF16)
        nc.vector.tensor_copy(a_bf, a_sb)
        pending.append((bi, x_lo, x_hi, a_bf))
        if len(pending) > 2:
            emit_ctx(*pending.pop(0))

    for args in pending:
        emit_ctx(*args)


def _install_ntff_hook():
    """Provide antenv.axon_hooks (missing in this image) so trace=True works.

    Mirrors trn_agent_boot's _ntff_profile_via_ctypes: drives NTFF capture
    through libaxon_pjrt.so's C ABI. Returns True if tracing is possible.
    """
    import os, types, ctypes, contextlib
    try:
        from antenv.axon_hooks import get_axon_ntff_profile_hook  # noqa: F401
        return True
    except ImportError:
        pass
    so_path = "/opt/axon/libaxon_pjrt.so"
    if not os.path.exists(so_path):
        return False
    lib = ctypes.CDLL(so_path)
    if not hasattr(lib, "axon_start_nrt_profile"):
        return False
    lib.axon_start_nrt_profile.argtypes = [ctypes.POINTER(ctypes.c_int64),
                                           ctypes.c_size_t]
    lib.axon_start_nrt_profile.restype = ctypes.c_int64
    lib.axon_stop_nrt_profile.argtypes = [ctypes.c_char_p]
    lib.axon_stop_nrt_profile.restype = ctypes.c_int64

    @contextlib.contextmanager
    def _hook(output_dir, device_ids):
        import jax
        jax.devices()
        if device_ids:
            ids = (ctypes.c_int64 * len(device_ids))(*device_ids)
            rc = lib.axon_start_nrt_profile(ids, len(device_ids))
        else:
            rc = lib.axon_start_nrt_profile(None, 0)
        if rc != 0:
            raise RuntimeError(f"axon_start_nrt_profile rc={rc}")
        try:
            yield
        finally:
            n = lib.axon_stop_nrt_profile(str(output_dir).encode())
            print(f"profile: {n} file(s) written to {output_dir}",
                  file=sys.stderr)

    mod = types.ModuleType("antenv.axon_hooks")
    mod.get_axon_ntff_profile_hook = lambda: _hook
    mod.set_axon_ntff_profile_hook = lambda h: None
    import antenv
    sys.modules["antenv.axon_hooks"] = mod
    antenv.axon_hooks = mod
    return True


_NC_CACHE = None


def _build():
    global _NC_CACHE
    if _NC_CACHE is not None:
        return _NC_CACHE
    nc = bacc.Bacc("TRN2", target_bir_lowering=False, debug=False)
    x = nc.dram_tensor("x", [BSH, T, D], F32, kind="ExternalInput")
    W = nc.dram_tensor("W", [D, U], F32, kind="ExternalInput")
    b = nc.dram_tensor("b", [U], F32, kind="ExternalInput")
    u = nc.dram_tensor("u", [U, 1], F32, kind="ExternalInput")
    octx = nc.dram_tensor("octx", [BSH, D], F32, kind="ExternalOutput")
    oattn = nc.dram_tensor("oattn", [BSH, P, NT], F32, kind="ExternalOutput")
    with tile.TileContext(nc) as tc:
        _body(tc, x.ap(), W.ap(), b.ap(), u.ap(), octx.ap(), oattn.ap())
    nc.compile()
    _NC_CACHE = nc
    return nc


def run(inputs, W, b, u, trace=False, tmpdir=None):
    if trace:
        trace = _install_ntff_hook()
    nc = _build()
    inputs = np.asarray(inputs, dtype=np.float32)
    W = np.asarray(W, dtype=np.float32)
    b = np.asarray(b, dtype=np.float32)
    u = np.asarray(u, dtype=np.float32)
    in_maps = [
        {"x": np.ascontiguousarray(inputs[BSH * i:BSH * (i + 1)]),
         "W": W, "b": b, "u": u}
        for i in range(NCORES)
    ]
    try:
        res = run_bass_kernel_spmd(nc, in_maps, core_ids=list(range(NCORES)),
                                   trace=trace, tmpdir=tmpdir)
    except Exception:
        # fallback: sequential single-core execution of each shard
        results = []
        for m in in_maps:
            r1 = run_bass_kernel_spmd(nc, [m], core_ids=[0], trace=False)
            results.append(r1.results[0])
            res = r1
        res.results = results
    ctx_full = np.concatenate([r["octx"] for r in res.results], axis=0)
    # [BSH, P, NT] -> [BSH, T, 1] with t = n * P + p
    attn_full = np.concatenate(
        [r["oattn"].transpose(0, 2, 1).reshape(BSH, T, 1) for r in res.results],
        axis=0)
    return (ctx_full, attn_full), res


def kernel(inputs, W, b, u):
    (ctx_full, attn_full), _ = run(inputs, W, b, u, trace=False)
    return ctx_full, attn_full


# revision 31
# speedup vs baseline: 1.2133x; 1.2133x over previous
"""Trainium2 Bass kernel for nn_AttentionLayer (attention pooling).

reference:
    score  = tanh(x @ W + b)          # [B,T,U]
    logits = score @ u                # [B,T,1]
    a      = softmax(logits, axis=T)  # [B,T,1]
    ctx    = sum_t x * a              # [B,D]
    returns (ctx, a)

Sharding: data-parallel over batch B=64 -> 8 cores x 8 batches.
Per-core pipeline (per batch):
    1. DMA x[b] (4096,256) fp32 -> SBUF bf16 [128,32,256]  (t on partitions)
    2. PE-transpose 64 x [128,128] tiles -> xT bf16 [128d, 2c, 32n, 128t]
    3. scoreT[u=64, t] = W.T @ xT   (PE, accumulate 2 d-chunks, 512-wide)
    4. tanh+bias on ACT (PSUM->SBUF, bf16)
    5. logits[t=128,32] : per t-chunk matmul (lhsT=scoreT strip, rhs=u)
    6. exp+rowsum on ACT; total via PE ones-matmul; reciprocal on DVE;
       broadcast via PE; a = e * invS on DVE
    7. ctx = sum_t a[t]*x[t,:] : PE accumulation (lhsT=a col, rhs=x tile)
"""
import sys
sys.path.insert(0, "/opt/trn_rl_repo")
import numpy as np
from contextlib import ExitStack

import concourse.bass as bass
import concourse.bacc as bacc
import concourse.tile as tile
from concourse import mybir
from concourse._compat import with_exitstack
from concourse.bass_utils import run_bass_kernel_spmd
from concourse.masks import make_identity

AF = mybir.ActivationFunctionType
F32 = mybir.dt.float32
BF16 = mybir.dt.float16  # 16-bit compute dtype (fp16: 10-bit mantissa)

P = 128          # partitions
T = 4096         # time steps
D = 256          # feature dim
U = 64           # attention units
NT = T // P      # 32 t-chunks of 128
NBLK = T // 512  # 8 t-blocks of 512
BSH = 8          # batches per core
NCORES = 8


@with_exitstack
def _body(ctx: ExitStack, tc: tile.TileContext, x, W, bvec, u, out_ctx, out_attn):
    nc = tc.nc
    consts = ctx.enter_context(tc.tile_pool(name="consts", bufs=1))
    # x alive from load(b) until ctx(b), emitted 2 batches later
    xpool = ctx.enter_context(tc.tile_pool(name="x", bufs=4))
    x32pool = ctx.enter_context(tc.tile_pool(name="x32", bufs=2))
    xtpool = ctx.enter_context(tc.tile_pool(name="xt", bufs=2))
    spool = ctx.enter_context(tc.tile_pool(name="scoreT", bufs=2))
    small = ctx.enter_context(tc.tile_pool(name="small", bufs=6))
    tps = ctx.enter_context(tc.tile_pool(name="tps", bufs=4, space="PSUM"))
    sps = ctx.enter_context(tc.tile_pool(name="sps", bufs=2, space="PSUM"))
    cps = ctx.enter_context(tc.tile_pool(name="cps", bufs=1, space="PSUM"))
    mps = ctx.enter_context(tc.tile_pool(name="mps", bufs=1, space="PSUM"))

    ident = consts.tile([P, P], BF16)
    make_identity(nc, ident)
    # plain fp32 loads (HWDGE) + on-chip casts: SWDGE cast-DMA avoided
    w32 = consts.tile([P, 2, U], F32)
    nc.sync.dma_start(out=w32, in_=W.rearrange("(c k) u -> k c u", k=P))
    w_sb = consts.tile([P, 2, U], BF16)
    nc.vector.tensor_copy(w_sb, w32)
    # u and b replicated into both 64-partition halves (for row/col tiling)
    u32 = consts.tile([P, 1], F32)
    nc.sync.dma_start(out=u32[0:U, :], in_=u)
    nc.sync.dma_start(out=u32[U:2 * U, :], in_=u)
    u_sb = consts.tile([P, 1], BF16)
    nc.vector.tensor_copy(u_sb, u32)
    b_sb = consts.tile([P, 1], F32)
    nc.sync.dma_start(out=b_sb[0:U, :], in_=bvec.rearrange("(u o) -> u o", o=1))
    nc.sync.dma_start(out=b_sb[U:2 * U, :], in_=bvec.rearrange("(u o) -> u o", o=1))
    ones_col = consts.tile([P, 1], F32)
    nc.vector.memset(ones_col, 1.0)
    ones_row = consts.tile([1, P], F32)
    nc.vector.memset(ones_row, 1.0)

    def emit_ctx(bi, x_lo, x_hi, a_bf):
        # ctx = sum_t a[t] * x[t,:]  (PE accumulate over 32 t-chunks)
        c_ps = cps.tile([1, D], F32)
        for n in range(NT):
            xt = x_lo[:, n, :] if n < NT // 2 else x_hi[:, n - NT // 2, :]
            nc.tensor.matmul(c_ps, a_bf[:, n:n + 1], xt,
                             start=(n == 0), stop=(n == NT - 1))
        ctx_sb = small.tile([1, D], F32)
        nc.scalar.activation(ctx_sb, c_ps, AF.Copy)
        nc.sync.dma_start(out=out_ctx[bi].rearrange("(o d) -> o d", o=1), in_=ctx_sb)

    pending = []  # (bi, x_sb, a_bf) awaiting ctx emission

    for bi in range(BSH):
        # 1. load x[bi] in two halves (earlier pipeline start) as plain fp32
        # (HWDGE), then cast to fp16 on ACT/DVE in quarters
        xre = x[bi].rearrange("(n p) d -> p n d", p=P)
        x_lo = xpool.tile([P, NT // 2, D], BF16, tag="xlo")
        x_hi = xpool.tile([P, NT // 2, D], BF16, tag="xhi")
        x32_lo = x32pool.tile([P, NT // 2, D], F32, tag="x32lo")
        x32_hi = x32pool.tile([P, NT // 2, D], F32, tag="x32hi")
        for h, (x16h, x32h, st) in enumerate([(x_lo, x32_lo, 0),
                                              (x_hi, x32_hi, NT // 2)]):
            nc.sync.dma_start(out=x32h, in_=xre[:, st:st + NT // 2, :])
            for q in range(2):
                sl = (slice(None), slice(q * (NT // 4), (q + 1) * (NT // 4)),
                      slice(None))
                if 2 * h + q == 3:
                    nc.vector.tensor_copy(x16h[sl], x32h[sl])
                else:
                    nc.scalar.activation(x16h[sl], x32h[sl], AF.Copy)

        def x_tile(n):
            return (x_lo[:, n, :] if n < NT // 2
                    else x_hi[:, n - NT // 2, :])

        # 2+3+4. interleaved per 512-block: transpose the 16 fp16 [128,128]
        # tiles it needs (each into its own per-tile PSUM bank — fp16 halves
        # the PE stream vs fp32 and the copy bytes), then the score matmuls
        # + tanh for that block.
        xT = xtpool.tile([P, 2, NT, P], BF16)
        scoreT = spool.tile([U, NBLK, 512], BF16)
        for blk in range(NBLK):
            for k in range(8):
                n, c = divmod(8 * blk + k, 2)
                tp = tps.tile([P, P], BF16)
                nc.tensor.transpose(tp, x_tile(n)[:, c * P:(c + 1) * P],
                                    ident)
                if k % 4 == 0:
                    nc.scalar.activation(xT[:, c, n, :], tp, AF.Copy)
                else:
                    nc.vector.tensor_copy(xT[:, c, n, :], tp)

            s_ps = sps.tile([U, 512], F32)
            for c in range(2):
                nc.tensor.matmul(s_ps, w_sb[:, c, :],
                                 xT[:, c, 4 * blk:4 * blk + 4, :],
                                 start=(c == 0), stop=(c == 1))
            nc.scalar.activation(scoreT[:, blk, :], s_ps, AF.Tanh,
                                 bias=b_sb[0:U, :])

        # 5. logits [128, 32]: column j = logits for t in [128j, 128j+128)
        l_ps = mps.tile([P, NT], F32, tag="misc")
        for j in range(NT):
            nc.tensor.matmul(l_ps[:, j:j + 1],
                             scoreT[:, j // 4, 128 * (j % 4):128 * (j % 4 + 1)],
                             u_sb[0:U, :], start=True, stop=True)

        # 6. softmax over all 4096 (no max-subtraction: |logits| <= 64*0.33*1)
        e_sb = small.tile([P, NT], F32)
        rowsum = small.tile([P, 1], F32)
        nc.scalar.activation(e_sb, l_ps, AF.Exp, accum_out=rowsum)
        s_ps1 = mps.tile([1, 1], F32, tag="misc")
        nc.tensor.matmul(s_ps1, rowsum, ones_col, start=True, stop=True)
        invS = small.tile([1, 1], F32)
        nc.vector.reciprocal(invS, s_ps1)
        ib_ps = mps.tile([P, 1], F32, tag="misc")
        nc.tensor.matmul(ib_ps, ones_row, invS, start=True, stop=True)
        invS_b = small.tile([P, 1], F32)
        nc.scalar.activation(invS_b, ib_ps, AF.Copy)
        a_sb = small.tile([P, NT], F32)
        nc.vector.tensor_scalar_mul(a_sb, e_sb, invS_b)
        # attn stored [P, NT] per batch (contiguous per-partition DMA);
        # host transposes to [T, 1] when unsharding.
        nc.sync.dma_start(out=out_attn[bi], in_=a_sb)

        # 7. ctx is emitted 2 batches late so the softmax tail never stalls
        # PE: ctx(b-2) matmuls fill PE while softmax(b) runs on ACT/DVE.
        a_bf = small.tile([P, NT], BF16)
        nc.vector.tensor_copy(a_bf, a_sb)
        pending.append((bi, x_lo, x_hi, a_bf))
        if len(pending) > 2:
            emit_ctx(*pending.pop(0))

    for args in pending:
        emit_ctx(*args)


def _install_ntff_hook():
    """Provide antenv.axon_hooks (missing in this image) so trace=True works.

    Mirrors trn_agent_boot's _ntff_profile_via_ctypes: drives NTFF capture
    through libaxon_pjrt.so's C ABI. Returns True if tracing is possible.
    """
    import os, types, ctypes, contextlib
    try:
        from antenv.axon_hooks import get_axon_ntff_profile_hook  # noqa: F401
        return True
    except ImportError:
        pass
    so_path = "/opt/axon/libaxon_pjrt.so"
    if not os.path.exists(so_path):
        return False
    lib = ctypes.CDLL(so_path)
    if not hasattr(lib, "axon_start_nrt_profile"):
        return False
    lib.axon_start_nrt_profile.argtypes = [ctypes.POINTER(ctypes.c_int64),
                                           ctypes.c_size_t]
    lib.axon_start_nrt_profile.restype = ctypes.c_int64
    lib.axon_stop_nrt_profile.argtypes = [ctypes.c_char_p]
    lib.axon_stop_nrt_profile.restype = ctypes.c_int64

    @contextlib.contextmanager
    def _hook(output_dir, device_ids):
        import jax
        jax.devices()
        if device_ids:
            ids = (ctypes.c_int64 * len(device_ids))(*device_ids)
            rc = lib.axon_start_nrt_profile(ids, len(device_ids))
        else:
            rc = lib.axon_start_nrt_profile(None, 0)
        if rc != 0:
            raise RuntimeError(f"axon_start_nrt_profile rc={rc}")
        try:
            yield
        finally:
            n = lib.axon_stop_nrt_profile(str(output_dir).encode())
            print(f"profile: {n} file(s) written to {output_dir}",
                  file=sys.stderr)

    mod = types.ModuleType("antenv.axon_hooks")
    mod.get_axon_ntff_profile_hook = lambda: _hook
    mod.set_axon_ntff_profile_hook = lambda h: None
    import antenv
    sys.modules["antenv.axon_hooks"] = mod
    antenv.axon_hooks = mod
    return True


_NC_CACHE = None


def _build():
    global _NC_CACHE
    if _NC_CACHE is not None:
        return _NC_CACHE
    nc = bacc.Bacc("TRN2", target_bir_lowering=False, debug=False)
    x = nc.dram_tensor("x", [BSH, T, D], F32, kind="ExternalInput")
    W = nc.dram_tensor("W", [D, U], F32, kind="ExternalInput")
    b = nc.dram_tensor("b", [U], F32, kind="ExternalInput")
    u = nc.dram_tensor("u", [U, 1], F32, kind="ExternalInput")
    octx = nc.dram_tensor("octx", [BSH, D], F32, kind="ExternalOutput")
    oattn = nc.dram_tensor("oattn", [BSH, P, NT], F32, kind="ExternalOutput")
    with tile.TileContext(nc) as tc:
        _body(tc, x.ap(), W.ap(), b.ap(), u.ap(), octx.ap(), oattn.ap())
    nc.compile()
    _NC_CACHE = nc
    return nc


def run(inputs, W, b, u, trace=False, tmpdir=None):
    if trace:
        trace = _install_ntff_hook()
    nc = _build()
    inputs = np.asarray(inputs, dtype=np.float32)
    W = np.asarray(W, dtype=np.float32)
    b = np.asarray(b, dtype=np.float32)
    u = np.asarray(u, dtype=np.float32)
    in_maps = [
        {"x": np.ascontiguousarray(inputs[BSH * i:BSH * (i + 1)]),
         "W": W, "b": b, "u": u}
        for i in range(NCORES)
    ]
    try:
        res = run_bass_kernel_spmd(nc, in_maps, core_ids=list(range(NCORES)),
                                   trace=trace, tmpdir=tmpdir)
    except Exception:
        # fallback: sequential single-core execution of each shard
        results = []
        for m in in_maps:
            r1 = run_bass_kernel_spmd(nc, [m], core_ids=[0], trace=False)
            results.append(r1.results[0])
            res = r1
        res.results = results
    ctx_full = np.concatenate([r["octx"] for r in res.results], axis=0)
    # [BSH, P, NT] -> [BSH, T, 1] with t = n * P + p
    attn_full = np.concatenate(
        [r["oattn"].transpose(0, 2, 1).reshape(BSH, T, 1) for r in res.results],
        axis=0)
    return (ctx_full, attn_full), res


def kernel(inputs, W, b, u):
    (ctx_full, attn_full), _ = run(inputs, W, b, u, trace=False)
    return ctx_full, attn_full


# revision 33
# speedup vs baseline: 1.6858x; 1.3894x over previous
"""Trainium2 Bass kernel for nn_AttentionLayer (attention pooling).

reference:
    score  = tanh(x @ W + b)          # [B,T,U]
    logits = score @ u                # [B,T,1]
    a      = softmax(logits, axis=T)  # [B,T,1]
    ctx    = sum_t x * a              # [B,D]
    returns (ctx, a)

Sharding: data-parallel over batch B=64 -> 8 cores x 8 batches.
Per-core pipeline (per batch):
    1. DMA x[b] (4096,256) fp32 -> SBUF bf16 [128,32,256]  (t on partitions)
    2. PE-transpose 64 x [128,128] tiles -> xT bf16 [128d, 2c, 32n, 128t]
    3. scoreT[u=64, t] = W.T @ xT   (PE, accumulate 2 d-chunks, 512-wide)
    4. tanh+bias on ACT (PSUM->SBUF, bf16)
    5. logits[t=128,32] : per t-chunk matmul (lhsT=scoreT strip, rhs=u)
    6. exp+rowsum on ACT; total via PE ones-matmul; reciprocal on DVE;
       broadcast via PE; a = e * invS on DVE
    7. ctx = sum_t a[t]*x[t,:] : PE accumulation (lhsT=a col, rhs=x tile)
"""
import sys
sys.path.insert(0, "/opt/trn_rl_repo")
import numpy as np
from contextlib import ExitStack

import concourse.bass as bass
import concourse.bacc as bacc
import concourse.tile as tile
from concourse import mybir
from concourse._compat import with_exitstack
from concourse.bass_utils import run_bass_kernel_spmd
from concourse.masks import make_identity

AF = mybir.ActivationFunctionType
F32 = mybir.dt.float32
BF16 = mybir.dt.float16  # 16-bit compute dtype (fp16: 10-bit mantissa)

P = 128          # partitions
T = 4096         # time steps
D = 256          # feature dim
U = 64           # attention units
NT = T // P      # 32 t-chunks of 128
NBLK = T // 512  # 8 t-blocks of 512
BSH = 8          # batches per core
NCORES = 8


@with_exitstack
def _body(ctx: ExitStack, tc: tile.TileContext, x, W, bvec, u, out_ctx, out_attn):
    nc = tc.nc
    consts = ctx.enter_context(tc.tile_pool(name="consts", bufs=1))
    # x alive from load(b) until ctx(b), emitted 2 batches later
    xpool = ctx.enter_context(tc.tile_pool(name="x", bufs=4))
    x32pool = ctx.enter_context(tc.tile_pool(name="x32", bufs=2))
    xtpool = ctx.enter_context(tc.tile_pool(name="xt", bufs=2))
    spool = ctx.enter_context(tc.tile_pool(name="scoreT", bufs=2))
    small = ctx.enter_context(tc.tile_pool(name="small", bufs=6))
    tps = ctx.enter_context(tc.tile_pool(name="tps", bufs=4, space="PSUM"))
    sps = ctx.enter_context(tc.tile_pool(name="sps", bufs=2, space="PSUM"))
    cps = ctx.enter_context(tc.tile_pool(name="cps", bufs=1, space="PSUM"))
    mps = ctx.enter_context(tc.tile_pool(name="mps", bufs=1, space="PSUM"))

    ident = consts.tile([P, P], BF16)
    make_identity(nc, ident)
    # plain fp32 loads (HWDGE) + on-chip casts: SWDGE cast-DMA avoided
    w32 = consts.tile([P, 2, U], F32)
    nc.sync.dma_start(out=w32, in_=W.rearrange("(c k) u -> k c u", k=P))
    w_sb = consts.tile([P, 2, U], BF16)
    nc.vector.tensor_copy(w_sb, w32)
    # u and b replicated into both 64-partition halves (for row/col tiling)
    u32 = consts.tile([P, 1], F32)
    nc.sync.dma_start(out=u32[0:U, :], in_=u)
    nc.sync.dma_start(out=u32[U:2 * U, :], in_=u)
    u_sb = consts.tile([P, 1], BF16)
    nc.vector.tensor_copy(u_sb, u32)
    b_sb = consts.tile([P, 1], F32)
    nc.sync.dma_start(out=b_sb[0:U, :], in_=bvec.rearrange("(u o) -> u o", o=1))
    nc.sync.dma_start(out=b_sb[U:2 * U, :], in_=bvec.rearrange("(u o) -> u o", o=1))
    ones_col = consts.tile([P, 1], F32)
    nc.vector.memset(ones_col, 1.0)
    ones_row = consts.tile([1, P], F32)
    nc.vector.memset(ones_row, 1.0)

    def emit_ctx(bi, x_lo, x_hi, a_bf):
        # ctx = sum_t a[t] * x[t,:]  (PE accumulate over 32 t-chunks)
        c_ps = cps.tile([1, D], F32)
        for n in range(NT):
            xt = x_lo[:, n, :] if n < NT // 2 else x_hi[:, n - NT // 2, :]
            nc.tensor.matmul(c_ps, a_bf[:, n:n + 1], xt,
                             start=(n == 0), stop=(n == NT - 1))
        ctx_sb = small.tile([1, D], F32)
        nc.scalar.activation(ctx_sb, c_ps, AF.Copy)
        nc.sync.dma_start(out=out_ctx[bi].rearrange("(o d) -> o d", o=1), in_=ctx_sb)

    pending = []  # (bi, x_sb, a_bf) awaiting ctx emission

    for bi in range(BSH):
        # 1. load x[bi] in two halves (earlier pipeline start) as plain fp32
        # (HWDGE), then cast to fp16 on ACT/DVE in quarters
        xre = x[bi].rearrange("(n p) d -> p n d", p=P)
        x_lo = xpool.tile([P, NT // 2, D], BF16, tag="xlo")
        x_hi = xpool.tile([P, NT // 2, D], BF16, tag="xhi")
        x32_lo = x32pool.tile([P, NT // 2, D], F32, tag="x32lo")
        x32_hi = x32pool.tile([P, NT // 2, D], F32, tag="x32hi")
        for h, (x16h, x32h, st) in enumerate([(x_lo, x32_lo, 0),
                                              (x_hi, x32_hi, NT // 2)]):
            nc.sync.dma_start(out=x32h, in_=xre[:, st:st + NT // 2, :])
            for q in range(2):
                sl = (slice(None), slice(q * (NT // 4), (q + 1) * (NT // 4)),
                      slice(None))
                if 2 * h + q == 3:
                    nc.vector.tensor_copy(x16h[sl], x32h[sl])
                else:
                    nc.scalar.activation(x16h[sl], x32h[sl], AF.Copy)

        def x_tile(n):
            return (x_lo[:, n, :] if n < NT // 2
                    else x_hi[:, n - NT // 2, :])

        # 2+3+4. interleaved: 4 fp16 transposes share one PSUM bank, one
        # wide strided copy drains them; then the score block pair that
        # consumes them. Even 512-blocks land on partitions 0:64, odd on
        # 64:128 (PE col-tiling -> concurrent matmul pairs; enables
        # row-tiled logits pairs below).
        xT = xtpool.tile([P, 2, NT, P], BF16)
        scoreT = spool.tile([U, NBLK, 512], BF16)
        for blk in range(NBLK):
            for g in range(2 * blk, 2 * blk + 2):
                tp = tps.tile([P, 4, P], BF16)
                for k in range(4):
                    n, c = divmod(4 * g + k, 2)
                    nc.tensor.transpose(tp[:, k, :],
                                        x_tile(n)[:, c * P:(c + 1) * P], ident)
                # tp[:, k, :] holds (n, c) = divmod(4g+k, 2); xT wants
                # [c, n]: xT[:, c, 2g + dn, :] = tp[:, 2*dn + c, :]
                dst = xT[:, :, 2 * g:2 * g + 2, :]
                src = tp.rearrange("p (n c) t -> p c n t", c=2)
                if g % 8 == 0:
                    nc.scalar.activation(dst, src, AF.Copy)
                else:
                    nc.vector.tensor_copy(dst, src)

            s_ps = sps.tile([U, 512], F32)
            for c in range(2):
                nc.tensor.matmul(s_ps, w_sb[:, c, :],
                                 xT[:, c, 4 * blk:4 * blk + 4, :],
                                 start=(c == 0), stop=(c == 1))
            nc.scalar.activation(scoreT[:, blk, :], s_ps, AF.Tanh,
                                 bias=b_sb[0:U, :])

        # 5. logits [128, 32]: column j = logits for t in [128j, 128j+128)
        l_ps = mps.tile([P, NT], F32, tag="misc")
        for j in range(NT):
            nc.tensor.matmul(l_ps[:, j:j + 1],
                             scoreT[:, j // 4, 128 * (j % 4):128 * (j % 4 + 1)],
                             u_sb[0:U, :], start=True, stop=True)

        # 6. softmax over all 4096 (no max-subtraction: |logits| <= 64*0.33*1)
        e_sb = small.tile([P, NT], F32)
        rowsum = small.tile([P, 1], F32)
        nc.scalar.activation(e_sb, l_ps, AF.Exp, accum_out=rowsum)
        s_ps1 = mps.tile([1, 1], F32, tag="misc")
        nc.tensor.matmul(s_ps1, rowsum, ones_col, start=True, stop=True)
        invS = small.tile([1, 1], F32)
        nc.vector.reciprocal(invS, s_ps1)
        ib_ps = mps.tile([P, 1], F32, tag="misc")
        nc.tensor.matmul(ib_ps, ones_row, invS, start=True, stop=True)
        invS_b = small.tile([P, 1], F32)
        nc.scalar.activation(invS_b, ib_ps, AF.Copy)
        a_sb = small.tile([P, NT], F32)
        nc.vector.tensor_scalar_mul(a_sb, e_sb, invS_b)
        # attn stored [P, NT] per batch (contiguous per-partition DMA);
        # host transposes to [T, 1] when unsharding.
        nc.sync.dma_start(out=out_attn[bi], in_=a_sb)

        # 7. ctx is emitted 2 batches late so the softmax tail never stalls
        # PE: ctx(b-2) matmuls fill PE while softmax(b) runs on ACT/DVE.
        a_bf = small.tile([P, NT], BF16)
        nc.vector.tensor_copy(a_bf, a_sb)
        pending.append((bi, x_lo, x_hi, a_bf))
        if len(pending) > 2:
            emit_ctx(*pending.pop(0))

    for args in pending:
        emit_ctx(*args)


def _install_ntff_hook():
    """Provide antenv.axon_hooks (missing in this image) so trace=True works.

    Mirrors trn_agent_boot's _ntff_profile_via_ctypes: drives NTFF capture
    through libaxon_pjrt.so's C ABI. Returns True if tracing is possible.
    """
    import os, types, ctypes, contextlib
    try:
        from antenv.axon_hooks import get_axon_ntff_profile_hook  # noqa: F401
        return True
    except ImportError:
        pass
    so_path = "/opt/axon/libaxon_pjrt.so"
    if not os.path.exists(so_path):
        return False
    lib = ctypes.CDLL(so_path)
    if not hasattr(lib, "axon_start_nrt_profile"):
        return False
    lib.axon_start_nrt_profile.argtypes = [ctypes.POINTER(ctypes.c_int64),
                                           ctypes.c_size_t]
    lib.axon_start_nrt_profile.restype = ctypes.c_int64
    lib.axon_stop_nrt_profile.argtypes = [ctypes.c_char_p]
    lib.axon_stop_nrt_profile.restype = ctypes.c_int64

    @contextlib.contextmanager
    def _hook(output_dir, device_ids):
        import jax
        jax.devices()
        if device_ids:
            ids = (ctypes.c_int64 * len(device_ids))(*device_ids)
            rc = lib.axon_start_nrt_profile(ids, len(device_ids))
        else:
            rc = lib.axon_start_nrt_profile(None, 0)
        if rc != 0:
            raise RuntimeError(f"axon_start_nrt_profile rc={rc}")
        try:
            yield
        finally:
            n = lib.axon_stop_nrt_profile(str(output_dir).encode())
            print(f"profile: {n} file(s) written to {output_dir}",
                  file=sys.stderr)

    mod = types.ModuleType("antenv.axon_hooks")
    mod.get_axon_ntff_profile_hook = lambda: _hook
    mod.set_axon_ntff_profile_hook = lambda h: None
    import antenv
    sys.modules["antenv.axon_hooks"] = mod
    antenv.axon_hooks = mod
    return True


_NC_CACHE = None


def _build():
    global _NC_CACHE
    if _NC_CACHE is not None:
        return _NC_CACHE
    nc = bacc.Bacc("TRN2", target_bir_lowering=False, debug=False)
    x = nc.dram_tensor("x", [BSH, T, D], F32, kind="ExternalInput")
    W = nc.dram_tensor("W", [D, U], F32, kind="ExternalInput")
    b = nc.dram_tensor("b", [U], F32, kind="ExternalInput")
    u = nc.dram_tensor("u", [U, 1], F32, kind="ExternalInput")
    octx = nc.dram_tensor("octx", [BSH, D], F32, kind="ExternalOutput")
    oattn = nc.dram_tensor("oattn", [BSH, P, NT], F32, kind="ExternalOutput")
    with tile.TileContext(nc) as tc:
        _body(tc, x.ap(), W.ap(), b.ap(), u.ap(), octx.ap(), oattn.ap())
    nc.compile()
    _NC_CACHE = nc
    return nc


def run(inputs, W, b, u, trace=False, tmpdir=None):
    if trace:
        trace = _install_ntff_hook()
    nc = _build()
    inputs = np.asarray(inputs, dtype=np.float32)
    W = np.asarray(W, dtype=np.float32)
    b = np.asarray(b, dtype=np.float32)
    u = np.asarray(u, dtype=np.float32)
    in_maps = [
        {"x": np.ascontiguousarray(inputs[BSH * i:BSH * (i + 1)]),
         "W": W, "b": b, "u": u}
        for i in range(NCORES)
    ]
    try:
        res = run_bass_kernel_spmd(nc, in_maps, core_ids=list(range(NCORES)),
                                   trace=trace, tmpdir=tmpdir)
    except Exception:
        # fallback: sequential single-core execution of each shard
        results = []
        for m in in_maps:
            r1 = run_bass_kernel_spmd(nc, [m], core_ids=[0], trace=False)
            results.append(r1.results[0])
            res = r1
        res.results = results
    ctx_full = np.concatenate([r["octx"] for r in res.results], axis=0)
    # [BSH, P, NT] -> [BSH, T, 1] with t = n * P + p
    attn_full = np.concatenate(
        [r["oattn"].transpose(0, 2, 1).reshape(BSH, T, 1) for r in res.results],
        axis=0)
    return (ctx_full, attn_full), res


def kernel(inputs, W, b, u):
    (ctx_full, attn_full), _ = run(inputs, W, b, u, trace=False)
    return ctx_full, attn_full
